# revision 38
# baseline (speedup 1.0000x reference)
"""Trainium2 Bass kernel for single-head causal attention.

Problem: B=4, N=2048, D=1024, f32.
  Q = x@Wq; K = x@Wk; V = x@Wv (biases are zero in this problem)
  S = Q K^T / sqrt(D), causal-masked, softmax over keys
  out = (softmax(S) @ V) @ Wo

Sharding: 8 cores = (4 batches) x (2 query-halves). Query rows are split in
512-row chunks; core half 0 takes chunks {0,3} of its batch, half 1 takes
{1,2} (both sum to 2560 causal key-tiles -> balanced). Causality differs per
half, so we build TWO specialized NEFFs and dispatch them concurrently on
disjoint 4-device meshes. No collectives needed (K/V are recomputed per
core-pair, which is cheaper than any cross-core traffic at this size).

All matmuls use float32r (FP32 storage, FP22 multiply) at free-dim >= 256
where the TensorEngine runs it at bf16 rate. Softmax skips the max
subtraction (scores are O(5) here, exp is safe in f32) so the denominator
can be accumulated with N=1 matmuls against a ones vector.
"""
import sys
import os

sys.path.insert(0, "/opt/trn_rl_repo")

import numpy as np

import concourse.bass as bass
import concourse.mybir as mybir
import concourse.tile as tile
from concourse import bacc
from concourse.masks import make_identity

P = 128
D = 1024
N = 2048
B = 4
NCORES = 8
F32 = mybir.dt.float32
F32R = mybir.dt.float32r
CHUNK = 512           # query-chunk width (free dim of S^T matmuls)
DSUB = D // P         # 8 feature sub-tiles
NSUB = N // P         # 16 row sub-tiles
NCHUNK = N // CHUNK   # 4 query chunks per batch
CHUNK_MAP = [(0, 3), (1, 2)]   # q-chunk indices per core-half
SCALE = 1.0 / np.sqrt(D)


def build(half: int, reps: int = 1, parts: str = 'all'):
    """Build the Bass graph for core-half `half` (0 or 1).

    reps > 1 wraps the whole body in a device-side loop - used only for
    wall-clock timing measurements (amortizes the host dispatch overhead).
    """
    chunks = CHUNK_MAP[half]
    nc = bacc.Bacc("TRN2", target_bir_lowering=False, debug=False,
                   enable_asserts=False, num_devices=NCORES // 2)

    x_d = nc.dram_tensor("x", [N, D], F32, kind="ExternalInput")
    w_d = {nm: nc.dram_tensor(nm, [D, D], F32, kind="ExternalInput")
           for nm in ("wq", "wk", "wv", "wo")}
    out_d = nc.dram_tensor("out", [2 * CHUNK, D], F32, kind="ExternalOutput")

    with tile.TileContext(nc) as tc:
        # ---- DRAM scratch ----
        with tc.tile_pool(name="dram", bufs=1, space="DRAM") as dram:
            kt_spill = dram.tile([DSUB, P, N], F32R)      # K^T: [d_sub][128d, n]
            v_spill = dram.tile([NSUB, P, D], F32R)       # V:   [n_sub][128n, d]
            qt_spill = None

            if reps > 1:
                with tc.For_i(0, reps, 1):
                    _build_body(nc, tc, half, chunks, x_d, w_d, out_d,
                                kt_spill, v_spill, qt_spill, parts)
            else:
                _build_body(nc, tc, half, chunks, x_d, w_d, out_d,
                            kt_spill, v_spill, qt_spill, parts)

    nc.compile()
    return nc


def _build_body(nc, tc, half, chunks, x_d, w_d, out_d, kt_spill, v_spill, qt_spill, parts='all'):
    from contextlib import ExitStack

    ctx = ExitStack()
    with ctx:
        const = ctx.enter_context(tc.tile_pool(name="const", bufs=1))

        # identity for PE transpose
        ident = const.tile([P, P], F32)
        make_identity(nc, ident)

        # ones column for denominator matmuls (memset can't write f32r;
        # round through a DVE copy instead)
        ones_f32 = const.tile([P, 2], F32)
        nc.gpsimd.memset(ones_f32, 1.0)
        ones = const.tile([P, 2], F32R)
        nc.vector.tensor_copy(ones, ones_f32)

        # zeros tile for blanking the trimmed region of diagonal P^T tiles
        zeros_f32 = const.tile([P, 2 * P], F32)
        nc.gpsimd.memset(zeros_f32, 0.0)

        # 4 diagonal mask tiles: M_m[k, q] = 1 if q >= 128*m + k else 0
        masks = []
        for m in range(4):
            mk = const.tile([P, CHUNK], F32, name=f"mask{m}")
            nc.gpsimd.memset(mk, 1.0)
            nc.gpsimd.affine_select(
                out=mk, in_=mk,
                compare_op=mybir.AluOpType.is_ge,
                fill=0.0,
                base=-P * m,
                channel_multiplier=-1,
                pattern=[[1, CHUNK]],
            )
            masks.append(mk)

        # Outer-scope pools: resident Q^T, K^T stream (first two 512-column
        # groups are produced straight into this pool and never leave SBUF),
        # and the V cache for key rows 0:512.
        qt_pool = ctx.enter_context(tc.tile_pool(name="qtp", bufs=16))
        kt_pool = ctx.enter_context(tc.tile_pool(name="ktp", bufs=16))
        v_pool = ctx.enter_context(tc.tile_pool(name="vp", bufs=4))

        qt_all = {}    # (ci, d_sub) -> resident Q^T tile
        kt_cache = {}  # (kch, d_sub) -> resident K^T tile for kch in {0, 1}

        # ================= Phase 1+2: x^T and projections =================
        ph12 = ExitStack()
        with ph12:
            xt_pool = ph12.enter_context(tc.tile_pool(name="xt", bufs=1))
            xld = ph12.enter_context(tc.tile_pool(name="xld", bufs=2))
            stg = ph12.enter_context(tc.tile_pool(name="stg", bufs=4))
            wpool = ph12.enter_context(tc.tile_pool(name="wp", bufs=8))
            tpsum = ph12.enter_context(
                tc.tile_pool(name="tpsum", bufs=2, space="PSUM"))
            ppsum = ph12.enter_context(
                tc.tile_pool(name="ppsum", bufs=6, space="PSUM"))

            xt = [xt_pool.tile([P, N], F32R, name=f"xt{i}", tag="xt", bufs=DSUB)
                  for i in range(DSUB)]

            def transpose_batch(jlist, split_first=False):
                for j in jlist:
                    xrow = xld.tile([P, D], F32, name="xrow", tag="xrow")
                    if split_first:
                        nc.sync.dma_start(xrow[:, :CHUNK],
                                          x_d.ap()[j * P:(j + 1) * P, :CHUNK])
                        nc.sync.dma_start(xrow[:, CHUNK:],
                                          x_d.ap()[j * P:(j + 1) * P, CHUNK:])
                        split_first = False
                    else:
                        nc.sync.dma_start(xrow, x_d.ap()[j * P:(j + 1) * P, :])
                    for i in range(DSUB):
                        tp = tpsum.tile([P, P], F32, name="tp", tag="tp")
                        nc.tensor.transpose(tp, xrow[:, i * P:(i + 1) * P], ident)
                        nc.vector.tensor_copy(xt[i][:, j * P:(j + 1) * P], tp)

            def load_w(nm, split=False):
                # weight DMAs go on the scalar-engine HWDGE queue so they
                # never head-block the x/K/V streams on the sync queue;
                # the first weight is split across both queues for latency
                tiles = []
                for s in range(DSUB):
                    wt = wpool.tile([P, D], F32R, name=f"{nm}{s}", tag="w")
                    eng = nc.sync if (split and s % 2 == 0) else nc.scalar
                    eng.dma_start(
                        wt, w_d[nm].ap().bitcast(F32R)[s * P:(s + 1) * P, :])
                    tiles.append(wt)
                return tiles

            transpose_batch(range(0, 4), split_first=True)
            if parts == 'ph1':
                transpose_batch(range(4, 16))
                # consume x^T so it isn't dead-code: one matmul + store
                with tc.tile_pool(name="sink", bufs=1, space="PSUM") as sk, \
                     tc.tile_pool(name="sinksb", bufs=1) as sksb:
                    pssink = sk.tile([P, CHUNK], F32, name="pssink")
                    for di in range(DSUB):
                        nc.tensor.matmul(pssink, lhsT=xt[di][:, :P],
                                         rhs=xt[di][:, :CHUNK],
                                         start=(di == 0), stop=(di == DSUB - 1))
                    snk = sksb.tile([P, CHUNK], F32, name="snk")
                    nc.vector.tensor_copy(snk, pssink)
                    nc.gpsimd.dma_start(out_d.ap()[0:P, 0:CHUNK], snk)
                return
            wk = load_w("wk", split=True)
            transpose_batch(range(4, 8))

            def kt_group(nch):
                for do in range(DSUB):
                    ps = ppsum.tile([P, CHUNK], F32, name="ktps", tag="pp")
                    for di in range(DSUB):
                        nc.tensor.matmul(
                            ps,
                            lhsT=wk[di][:, do * P:(do + 1) * P],
                            rhs=xt[di][:, nch * CHUNK:(nch + 1) * CHUNK],
                            start=(di == 0), stop=(di == DSUB - 1))
                    if nch < 2:
                        # keep in SBUF for phase 3 (no DRAM round trip)
                        kts = kt_pool.tile([P, CHUNK], F32R,
                                           name=f"ktc{nch}_{do}", tag="kt")
                        kt_cache[(nch, do)] = kts
                    else:
                        kts = stg.tile([P, CHUNK], F32R, name="kts", tag="kts",
                                       bufs=2)
                    nc.vector.tensor_copy(kts, ps)
                    if nch >= 2:
                        nc.sync.dma_start(
                            kt_spill[do][:, nch * CHUNK:(nch + 1) * CHUNK], kts)

            kt_group(0)
            transpose_batch(range(8, 12))
            kt_group(1)
            transpose_batch(range(12, 16))
            kt_group(2)
            kt_group(3)

            # --- Q^T for my chunks (scaled by 1/sqrt(D)); stays in SBUF ---
            wq = load_w("wq")
            for ci, qc in enumerate(chunks):
                g0 = qc * CHUNK
                for do in range(DSUB):
                    ps = ppsum.tile([P, CHUNK], F32, name="qtps", tag="pp")
                    for di in range(DSUB):
                        nc.tensor.matmul(
                            ps,
                            lhsT=wq[di][:, do * P:(do + 1) * P],
                            rhs=xt[di][:, g0:g0 + CHUNK],
                            start=(di == 0), stop=(di == DSUB - 1))
                    qts = qt_pool.tile([P, CHUNK], F32R,
                                       name=f"qt{ci}_{do}", tag="qt")
                    nc.vector.tensor_scalar_mul(qts, ps, float(SCALE))
                    qt_all[(ci, do)] = qts

            # --- V: [n, d] = x V ; rows 0:512 cached, rest spilled ---
            wv = load_w("wv")
            for ns in range(NSUB):
                vstg = stg.tile([P, D], F32R, name="vstg", tag="vstg", bufs=2)
                for dh in range(2):
                    ps = ppsum.tile([P, CHUNK], F32, name="vps", tag="pp")
                    for di in range(DSUB):
                        nc.tensor.matmul(
                            ps,
                            lhsT=xt[di][:, ns * P:(ns + 1) * P],
                            rhs=wv[di][:, dh * CHUNK:(dh + 1) * CHUNK],
                            start=(di == 0), stop=(di == DSUB - 1))
                    nc.vector.tensor_copy(
                        vstg[:, dh * CHUNK:(dh + 1) * CHUNK], ps)
                nc.sync.dma_start(v_spill[ns], vstg)

        # ================= Phase 3: attention per q-chunk =================
        ph3 = ExitStack()
        with ph3:
            pt_pool = ph3.enter_context(tc.tile_pool(name="ptp", bufs=16))
            wo_pool = ph3.enter_context(tc.tile_pool(name="wop", bufs=1))
            att_pool = ph3.enter_context(tc.tile_pool(name="attp", bufs=16))
            kts_pool = ph3.enter_context(tc.tile_pool(name="ktsp", bufs=8))
            out_pool = ph3.enter_context(tc.tile_pool(name="outp", bufs=2))
            dn_pool = ph3.enter_context(tc.tile_pool(name="dnp", bufs=2))

            wo = []
            for s in range(DSUB):
                wt = wo_pool.tile([P, D], F32R, name=f"wo{s}", tag="wo", bufs=DSUB)
                nc.gpsimd.dma_start(
                    wt, w_d["wo"].ap().bitcast(F32R)[s * P:(s + 1) * P, :])
                wo.append(wt)

            def pass_a(ci, qc):
                """S^T -> exp -> P^T tiles + denominator. Returns (pt_tiles, recip)."""
                T = 4 * (qc + 1)
                KCH = T // 4
                qt = [qt_all[(ci, s)] for s in range(DSUB)]
                pt_tiles = []
                pa_psum = ExitStack()
                with pa_psum:
                    spsum = pa_psum.enter_context(
                        tc.tile_pool(name="spsum", bufs=2, space="PSUM"))
                    dpsum = pa_psum.enter_context(
                        tc.tile_pool(name="dpsum", bufs=1, space="PSUM"))

                    dn_ps = dpsum.tile([1, CHUNK], F32, name="dnrow", tag="dnrow")

                    ktc_tiles = {}
                    for kch in range(KCH):
                        if kch < 2:
                            ktc_tiles[kch] = [kt_cache[(kch, s)]
                                              for s in range(DSUB)]
                            continue
                        ktc = []
                        for s in range(DSUB):
                            kt = kts_pool.tile([P, CHUNK], F32R,
                                               name=f"kt{s}", tag="kts")
                            nc.sync.dma_start(
                                kt, kt_spill[s][:, kch * CHUNK:(kch + 1) * CHUNK])
                            ktc.append(kt)
                        ktc_tiles[kch] = ktc

                    for kc in range(T):
                        ktc = ktc_tiles[kc // 4]
                        col = (kc % 4) * P
                        m = kc - 4 * qc
                        # block-causal: diagonal tile m only touches query
                        # columns >= 128*m (cap at 256 so N' stays >= 256,
                        # where fp32r runs at full rate)
                        off = min(P * m, 2 * P) if m > 0 else 0
                        ps = spsum.tile([P, CHUNK], F32, name="sps", tag="sp")
                        for s in range(DSUB):
                            nc.tensor.matmul(
                                ps[:, off:],
                                lhsT=ktc[s][:, col:col + P],
                                rhs=qt[s][:, off:],
                                start=(s == 0), stop=(s == DSUB - 1))
                        pt = pt_pool.tile([P, CHUNK], F32R, name="pt", tag="pt")
                        nc.scalar.activation(
                            pt[:, off:], ps[:, off:],
                            mybir.ActivationFunctionType.Exp)
                        if 0 <= m < 4:
                            nc.vector.tensor_mul(pt[:, off:], pt[:, off:],
                                                 masks[m][:, off:])
                        pt_tiles.append((pt, off))
                        # denominator row: dn[0, q] += sum_k P^T[k, q]
                        # (ones as the stationary operand -> 1-column weight
                        # load, full-rate N=512 streaming). The trimmed region
                        # of diagonal tiles is zeroed so every tile contributes
                        # full width and the PSUM chain stays uniform.
                        if off > 0:
                            nc.vector.tensor_copy(pt[:, :off],
                                                  zeros_f32[:, :off])
                        nc.tensor.matmul(
                            dn_ps,
                            lhsT=ones[:, 0:1],
                            rhs=pt,
                            start=(kc == 0), stop=(kc == T - 1))

                    dn_row = dn_pool.tile([1, CHUNK], F32, name="dnrow_sb",
                                          tag="dnrow_sb", bufs=1)
                    nc.vector.tensor_copy(dn_row, dn_ps)
                    # transpose [1, 512] -> 4x [128, 1] via PE transpose-mode
                    dn_sb = dn_pool.tile([P, 4], F32, name="dnsb", tag="dnsb")
                    for qs in range(4):
                        tp = dpsum.tile([P, 2], F32, name="dtp", tag="dtp")
                        nc.tensor.transpose(
                            tp[:, 0:1], dn_row[0:1, qs * P:(qs + 1) * P],
                            ident[0:1, 0:1])
                        nc.vector.tensor_copy(dn_sb[:, qs:qs + 1], tp[:, 0:1])
                recip = dn_pool.tile([P, 4], F32, name="recip", tag="recip")
                nc.vector.reciprocal(recip, dn_sb)
                return pt_tiles, recip

            def pass_b(ci, qc, pt_tiles):
                """att^T[do] = sum_k V^T P^T. Returns att tiles."""
                T = 4 * (qc + 1)
                att = []
                pb_psum = ExitStack()
                with pb_psum:
                    apsum = pb_psum.enter_context(
                        tc.tile_pool(name="apsum", bufs=1, space="PSUM"))
                    a_ps = [apsum.tile([P, CHUNK], F32, name=f"a{do}", tag=f"a{do}")
                            for do in range(DSUB)]
                    for kc in range(T):
                        vt = v_pool.tile([P, D], F32R, name="vt", tag="vt")
                        nc.sync.dma_start(vt, v_spill[kc])
                        pt, off = pt_tiles[kc]
                        for do in range(DSUB):
                            nc.tensor.matmul(
                                a_ps[do][:, off:],
                                lhsT=vt[:, do * P:(do + 1) * P],
                                rhs=pt[:, off:],
                                start=(kc == 0), stop=(kc == T - 1))
                    for do in range(DSUB):
                        at = att_pool.tile([P, CHUNK], F32R,
                                           name=f"att{do}", tag="att")
                        if do % 2 == 0:
                            nc.vector.tensor_copy(at, a_ps[do])
                        else:
                            nc.scalar.activation(
                                at, a_ps[do], mybir.ActivationFunctionType.Copy)
                        att.append(at)
                return att

            def pass_c(ci, qc, att, recip):
                """out = (att^T)^T Wo / denom -> DRAM."""
                pc_psum = ExitStack()
                with pc_psum:
                    opsum = pc_psum.enter_context(
                        tc.tile_pool(name="opsum", bufs=2, space="PSUM"))
                    for qs in range(4):
                        for dh in range(2):
                            ps = opsum.tile([P, CHUNK], F32, name="ops", tag="op")
                            for s in range(DSUB):
                                nc.tensor.matmul(
                                    ps,
                                    lhsT=att[s][:, qs * P:(qs + 1) * P],
                                    rhs=wo[s][:, dh * CHUNK:(dh + 1) * CHUNK],
                                    start=(s == 0), stop=(s == DSUB - 1))
                            ot = out_pool.tile([P, CHUNK], F32,
                                               name="ot", tag="ot")
                            if dh == 0:
                                nc.vector.tensor_scalar_mul(
                                    ot, ps, recip[:, qs:qs + 1])
                            else:
                                nc.scalar.activation(
                                    ot, ps, mybir.ActivationFunctionType.Copy,
                                    scale=recip[:, qs:qs + 1])
                            nc.gpsimd.dma_start(
                                out_d.ap()[ci * CHUNK + qs * P:
                                           ci * CHUNK + (qs + 1) * P,
                                           dh * CHUNK:(dh + 1) * CHUNK],
                                ot)

            if parts == 'ph12':
                return

            # Interleave chunk passes so the other chunk's matmuls hide the
            # PSUM->SBUF copy latencies at each pass boundary.
            c0, c1 = chunks[0], chunks[1]
            pt0, recip0 = pass_a(0, c0)
            att0 = pass_b(0, c0, pt0)
            pt1, recip1 = pass_a(1, c1)
            pass_c(0, c0, att0, recip0)
            att1 = pass_b(1, c1, pt1)
            pass_c(1, c1, att1, recip1)

# ---------------------------------------------------------------------------
# Host-side dispatch
# ---------------------------------------------------------------------------

_CACHE = {}


def _get_executables():
    if "exes" in _CACHE:
        return _CACHE["exes"]
    import jax
    from jax.sharding import Mesh, PartitionSpec
    from jax.experimental.shard_map import shard_map
    from concourse.bass2jax import (_bass_exec_p, install_neuronx_cc_hook,
                                    partition_id_tensor)

    install_neuronx_cc_hook()
    devices = jax.devices()
    assert len(devices) >= NCORES, f"need {NCORES} devices, have {len(devices)}"

    exes = []
    for half in range(2):
        nc = build(half)
        partition_name = (nc.partition_id_tensor.name
                          if nc.partition_id_tensor else None)
        in_names, out_names, out_avals, zero_shapes = [], [], [], []
        for alloc in nc.m.functions[0].allocations:
            if not isinstance(alloc, mybir.MemoryLocationSet):
                continue
            name = alloc.memorylocations[0].name
            if alloc.kind == "ExternalInput":
                if name != partition_name:
                    in_names.append(name)
            elif alloc.kind == "ExternalOutput":
                out_names.append(name)
                shape = tuple(alloc.tensor_shape)
                dtype = mybir.dt.np(alloc.dtype)
                out_avals.append(jax.core.ShapedArray(shape, dtype))
                zero_shapes.append((shape, dtype))
        n_params = len(in_names)
        all_in_names = list(in_names) + list(out_names)
        if partition_name is not None:
            all_in_names.append(partition_name)
        donate = tuple(range(n_params, n_params + len(out_names)))

        def _body(*args, _nc=nc, _out_avals=tuple(out_avals),
                  _all_in=tuple(all_in_names), _out=tuple(out_names),
                  _pid=partition_name):
            operands = list(args)
            if _pid is not None:
                operands.append(partition_id_tensor())
            return tuple(_bass_exec_p.bind(
                *operands, out_avals=_out_avals, in_names=_all_in,
                out_names=_out, lowering_input_output_aliases=(),
                sim_require_finite=True, sim_require_nnan=True, nc=_nc))

        devs = devices[half * 4:(half + 1) * 4]
        mesh = Mesh(np.asarray(devs), ("core",))
        in_specs = (PartitionSpec("core"),) * (n_params + len(out_names))
        out_specs = (PartitionSpec("core"),) * len(out_names)
        sharded = jax.jit(
            shard_map(_body, mesh=mesh, in_specs=in_specs,
                      out_specs=out_specs, check_rep=False),
            donate_argnums=donate, keep_unused=True)
        exes.append(dict(fn=sharded, in_names=in_names,
                         out_names=out_names, zero_shapes=zero_shapes))
    _CACHE["exes"] = exes
    return exes


def kernel(**inputs):
    x = np.asarray(inputs["x"], dtype=np.float32)      # [B, N, D]
    Wq = np.asarray(inputs["Wq"], dtype=np.float32)
    Wk = np.asarray(inputs["Wk"], dtype=np.float32)
    Wv = np.asarray(inputs["Wv"], dtype=np.float32)
    Wo = np.asarray(inputs["Wo"], dtype=np.float32)

    exes = _get_executables()
    outs = []
    for half in range(2):
        ex = exes[half]
        per_core = []
        for b in range(B):
            m = {"x": x[b], "wq": Wq, "wk": Wk, "wv": Wv, "wo": Wo}
            per_core.append([np.ascontiguousarray(m[nm]) for nm in ex["in_names"]])
        concat_in = [np.concatenate([per_core[c][i] for c in range(B)], axis=0)
                     for i in range(len(ex["in_names"]))]
        zeros = [np.zeros((B * s[0], *s[1:]), dt) for s, dt in ex["zero_shapes"]]
        outs.append(ex["fn"](*concat_in, *zeros))

    import jax
    jax.block_until_ready(outs)

    out_full = np.empty((B, N, D), dtype=np.float32)
    for half in range(2):
        ex = exes[half]
        arr = np.asarray(outs[half][ex["out_names"].index("out")])
        arr = arr.reshape(B, 2 * CHUNK, D)
        for b in range(B):
            for ci, qc in enumerate(CHUNK_MAP[half]):
                out_full[b, qc * CHUNK:(qc + 1) * CHUNK] = \
                    arr[b, ci * CHUNK:(ci + 1) * CHUNK]
    return out_full
